# revision 3
# baseline (speedup 1.0000x reference)
"""Causal single-head attention (B=8, T=2048, D=128, H=16) on 8 Trainium2 cores.

Strategy: data-parallel over batch (1 batch element per NeuronCore). Per core:
  - Load x [T, D], PE-transpose to xT [D, T].
  - Project qT/kT = (Wq/Wk)^T @ xT with head dim zero-padded 16->128 so every
    matmul contracts over K=128; v tiles [128, 17] with a ones column so the
    softmax denominator falls out of the PV matmul for free.
  - Scores are computed TRANSPOSED: ST[keys, queries] = kT_j^T @ qT_block, so
    exp(ST) (ACT, scale=1/4 folded in) is directly the PV stationary-side
    operand -- no per-tile transposes of the probability matrix.
  - PV: O'T[17, W] += V'_j^T @ PT_j accumulated in PSUM over key tiles.
  - Causal masking: only key tiles j with 128*j < W*(qb+1) are computed; the
    two diagonal tiles get a multiplicative 0/1 mask after exp.
Output per core: outT [17, T] (16 unnormalized head dims + the exp-sum row).
Host divides and transposes during the gather step.
"""

import os

import numpy as np

B, T, D, H = 8, 2048, 128, 16
NT = T // 128        # 16 key tiles of 128
W = 256              # query block width (fp32r needs moving dim >= 256)
NQB = T // W         # 8 query blocks
GROUP = 4            # key tiles exp'd per ACT call ([128, GROUP*W] <= 2 PSUM banks)
SCALE = H ** -0.5

_CACHE = {}


def _build(prec: str):
    import concourse.mybir as mybir
    import concourse.tile as tile
    from concourse import bacc

    f32 = mybir.dt.float32
    mm_dt = mybir.dt.float32r if prec == "f32r" else f32
    Exp = mybir.ActivationFunctionType.Exp

    nc = bacc.Bacc()
    x = nc.declare_dram_parameter("x", [T, D], f32, isOutput=False)
    wq = nc.declare_dram_parameter("wq", [D, 128], mm_dt, isOutput=False)
    wk = nc.declare_dram_parameter("wk", [D, 128], mm_dt, isOutput=False)
    wv = nc.declare_dram_parameter("wv", [D, H], mm_dt, isOutput=False)
    ident = nc.declare_dram_parameter("ident", [128, 128], f32, isOutput=False)
    dmask = nc.declare_dram_parameter("dmask", [128, 2 * W], f32, isOutput=False)
    outT = nc.declare_dram_parameter("outT", [H + 1, T], f32, isOutput=True)

    with tile.TileContext(nc) as tc:
        with tc.tile_pool(name="sb", bufs=1) as sb:
            # ---- persistent SBUF buffers ----
            wq_sb = sb.tile([D, 128], mm_dt, tag="wq")
            wk_sb = sb.tile([D, 128], mm_dt, tag="wk")
            wv_sb = sb.tile([D, H], mm_dt, tag="wv")
            id_sb = sb.tile([128, 128], f32, tag="ident")
            dm_sb = sb.tile([128, 2 * W], f32, tag="dmask")
            nc.sync.dma_start(wq_sb[:], wq.ap())
            nc.sync.dma_start(wk_sb[:], wk.ap())
            nc.sync.dma_start(wv_sb[:], wv.ap())
            nc.sync.dma_start(id_sb[:], ident.ap())
            nc.sync.dma_start(dm_sb[:], dmask.ap())

            xin = sb.tile([128, NT, D], f32, tag="xin")       # x tiles, natural
            nc.sync.dma_start(xin[:], x.ap().rearrange("(n p) d -> p n d", p=128))

            xT = sb.tile([128, NT, 128], mm_dt, tag="xT")     # [d, tile, t]
            qT = sb.tile([128, T], mm_dt, tag="qT")           # [head(pad), t]
            kT = sb.tile([128, T], mm_dt, tag="kT")
            vS = sb.tile([128, NT, H + 1], mm_dt, tag="vS")   # [t, tile, head+1]
            oT = sb.tile([H + 1, T], f32, tag="oT")

            # ---- phase 1: transpose x, project q/k/v ----
            with tc.tile_pool(name="psA", bufs=2, space="PSUM") as psA:
                for i in range(NT):
                    tp = psA.tile([128, 128], f32, tag="tp")
                    nc.tensor.transpose(tp[:], xin[:, i, :], id_sb[:])
                    nc.vector.tensor_copy(xT[:, i, :], tp[:])

                for g in range(T // 1024):  # 2 chunks of 1024 for q and k
                    pq = psA.tile([128, 1024], f32, tag="proj")
                    for h in range(2):
                        nc.tensor.matmul(
                            pq[:, h * 512:(h + 1) * 512],
                            wq_sb[:],
                            xT[:, 8 * g + 4 * h: 8 * g + 4 * h + 4, :],
                        )
                    nc.scalar.copy(qT[:, 1024 * g:1024 * (g + 1)], pq[:])
                    pk = psA.tile([128, 1024], f32, tag="proj")
                    for h in range(2):
                        nc.tensor.matmul(
                            pk[:, h * 512:(h + 1) * 512],
                            wk_sb[:],
                            xT[:, 8 * g + 4 * h: 8 * g + 4 * h + 4, :],
                        )
                    nc.vector.tensor_copy(kT[:, 1024 * g:1024 * (g + 1)], pk[:])

                pv = psA.tile([128, NT, H], f32, tag="vproj")
                for i in range(NT):
                    nc.tensor.matmul(pv[:, i, :], xT[:, i, :], wv_sb[:])
                nc.vector.tensor_copy(vS[:, :, :H], pv[:])
                nc.vector.memset(vS[:, :, H].bitcast(f32), 1.0)

            # ---- phase 2: attention per query block ----
            with (
                tc.tile_pool(name="psS", bufs=2, space="PSUM") as psS,
                tc.tile_pool(name="psO", bufs=2, space="PSUM") as psO,
                tc.tile_pool(name="pt", bufs=3) as ptp,
            ):
                for qb in range(NQB):
                    nj = (W * (qb + 1)) // 128  # causal key tiles
                    o_ps = psO.tile([H + 1, W], f32, tag="o")
                    for g0 in range(0, nj, GROUP):
                        gn = min(GROUP, nj - g0)
                        st = psS.tile([128, GROUP * W], f32, tag="st")
                        for jj in range(gn):
                            j = g0 + jj
                            nc.tensor.matmul(
                                st[:, jj * W:(jj + 1) * W],
                                kT[:, 128 * j:128 * (j + 1)],
                                qT[:, W * qb:W * (qb + 1)],
                            )
                        pt = ptp.tile([128, GROUP * W], mm_dt, tag="pt")
                        nc.scalar.activation(
                            pt[:, :gn * W], st[:, :gn * W], Exp, scale=SCALE
                        )
                        if g0 + gn == nj:  # group holding the 2 diagonal tiles
                            off = (nj - 2 - g0) * W
                            nc.vector.tensor_mul(
                                pt[:, off:off + 2 * W],
                                pt[:, off:off + 2 * W],
                                dm_sb[:],
                            )
                        for jj in range(gn):
                            j = g0 + jj
                            nc.tensor.matmul(
                                o_ps[:],
                                vS[:, j, :],
                                pt[:, jj * W:(jj + 1) * W],
                                start=(j == 0),
                                stop=(j == nj - 1),
                            )
                    nc.vector.tensor_copy(oT[:, W * qb:W * (qb + 1)], o_ps[:])

            nc.sync.dma_start(outT.ap(), oT[:])

    nc.finalize()
    return nc


def _get_nc(prec: str):
    if prec not in _CACHE:
        _CACHE[prec] = _build(prec)
    return _CACHE[prec]


def _host_inputs(Wq, Wk, Wv):
    wq_p = np.zeros((D, 128), np.float32)
    wq_p[:, :H] = Wq
    wk_p = np.zeros((D, 128), np.float32)
    wk_p[:, :H] = Wk
    ident = np.eye(128, dtype=np.float32)
    r = np.arange(128)[:, None]
    c = np.arange(W)[None, :]
    m1 = (r <= c).astype(np.float32)
    m2 = (128 + r <= c).astype(np.float32)
    dmask = np.concatenate([m1, m2], axis=1)
    return wq_p, wk_p, np.ascontiguousarray(Wv, np.float32), ident, dmask


def kernel(inpEmb, Wq, Wk, Wv):
    from concourse.bass_utils import run_bass_kernel_spmd

    prec = os.environ.get("ATT_PREC", "f32r")
    nc = _get_nc(prec)
    wq_p, wk_p, wv_c, ident, dmask = _host_inputs(Wq, Wk, Wv)
    x = np.ascontiguousarray(inpEmb, dtype=np.float32)
    in_maps = [
        {"x": x[b], "wq": wq_p, "wk": wk_p, "wv": wv_c, "ident": ident, "dmask": dmask}
        for b in range(B)
    ]
    br = run_bass_kernel_spmd(nc, in_maps, list(range(B)))
    out = np.empty((B, T, H), np.float32)
    for b in range(B):
        oT = br.results[b]["outT"]
        out[b] = (oT[:H] / oT[H:H + 1]).T
    return out


# revision 16
# speedup vs baseline: 1.1261x; 1.1261x over previous
"""Causal single-head attention (B=8, T=2048, D=128, H=16) on 8 Trainium2 cores.

Strategy: data-parallel over batch (1 batch element per NeuronCore). Per core:
  - x arrives pre-transposed from the host as xT [D, T] (contiguous DMA).
  - Project qT/kT = (Wq/Wk)^T @ xT with head dim zero-padded 16->128 so every
    matmul contracts over K=128; v tiles [128, 17] carry a ones column so the
    softmax denominator falls out of the PV matmul for free.
  - Scores are computed TRANSPOSED: ST[keys, queries] = kT_j^T @ qT_block, so
    exp(ST) (ACT, scale=1/4 folded in) is directly the PV stationary-side
    operand -- no per-tile transposes of the probability matrix.
  - PV: O'T[17, W] += V'_j^T @ PT_j accumulated in PSUM over key tiles.
  - Causal masking: only key tiles j with 128*j < W*(qb+1) are computed; the
    two diagonal tiles get a multiplicative 0/1 mask after exp.
  - ST-matmul groups are software-pipelined one group ahead of the PV matmuls
    so TensorE streams scores for group g+1 while ScalarE exponentiates group
    g and TensorE then immediately consumes it for PV.
Output per core: outT [17, T] (16 unnormalized head dims + the exp-sum row).
Host divides and transposes during the gather step.
"""

import os

import numpy as np

B, T, D, H = 8, 2048, 128, 16
NT = T // 128        # 16 key tiles of 128
W = 256              # query block width (fp32r needs moving dim >= 256)
NQB = T // W         # 8 query blocks
GROUP = 4            # key tiles per exp call ([128, GROUP*W] = 2 PSUM banks)
SCALE = H ** -0.5

_CACHE = {}


def _build(prec: str):
    import concourse.mybir as mybir
    import concourse.tile as tile
    from concourse import bacc

    f32 = mybir.dt.float32
    mm_dt = mybir.dt.float32r if prec == "f32r" else f32
    Exp = mybir.ActivationFunctionType.Exp

    nc = bacc.Bacc()
    xT_d = nc.declare_dram_parameter("xT", [D, T], mm_dt, isOutput=False)
    # packed constants: wq[0:128] | wk[128:256] | wv[256:272] | dmask[272:784]
    cst = nc.declare_dram_parameter("cst", [128, 784], mm_dt, isOutput=False)
    outT = nc.declare_dram_parameter("outT", [H + 1, T], f32, isOutput=True)

    with tile.TileContext(nc) as tc:
        with tc.tile_pool(name="sb", bufs=1) as sb:
            # ---- persistent SBUF buffers ----
            cst_sb = sb.tile([128, 784], mm_dt, tag="cst")
            nc.gpsimd.dma_start(cst_sb[:], cst.ap())  # SWDGE: parallel to x
            wq_sb = cst_sb[:, 0:128]
            wk_sb = cst_sb[:, 128:256]
            wv_sb = cst_sb[:, 256:272]
            dm_sb = cst_sb[:, 272:784].bitcast(f32)

            xT = sb.tile([128, T], mm_dt, tag="xT")           # [d, t]
            for c in range(2):
                nc.sync.dma_start(
                    xT[:, 1024 * c:1024 * (c + 1)],
                    xT_d.ap()[:, 1024 * c:1024 * (c + 1)],
                )

            warm = sb.tile([1, 2], f32, tag="warm")
            nc.vector.memset(warm[:, 0:1], 0.0)
            nc.scalar.activation(warm[:, 1:2], warm[:, 0:1], Exp)

            qTc = [sb.tile([128, 512], mm_dt, tag=f"qT{g}", name=f"qT{g}") for g in range(4)]
            kTc = [sb.tile([128, 512], mm_dt, tag=f"kT{g}", name=f"kT{g}") for g in range(4)]
            vSc = [sb.tile([128, 4, H + 1], mm_dt, tag=f"vS{g}", name=f"vS{g}") for g in range(4)]
            oTc = [sb.tile([H + 1, W], f32, tag=f"oT{qb}", name=f"oT{qb}") for qb in range(NQB)]

            groups = []
            for qb in range(NQB):
                nj = (W * (qb + 1)) // 128
                for g0 in range(0, nj, GROUP):
                    groups.append((qb, nj, g0, min(GROUP, nj - g0)))

            with (
                tc.tile_pool(name="psS", bufs=2, space="PSUM") as psS,
                tc.tile_pool(name="psO", bufs=2, space="PSUM") as psO,
                tc.tile_pool(name="psA", bufs=2, space="PSUM") as psA,
                tc.tile_pool(name="pt", bufs=6) as ptp,
            ):
                o_tiles = {}
                pt_tiles = {}

                def emit_proj(g):
                    sl = slice(512 * g, 512 * (g + 1))
                    pk = psA.tile([128, 512], f32, tag="proj", name=f"pk{g}")
                    nc.tensor.matmul(pk[:], wk_sb[:], xT[:, sl])
                    nc.vector.tensor_copy(kTc[g][:], pk[:])
                    pq = psA.tile([128, 512], f32, tag="proj", name=f"pq{g}")
                    nc.tensor.matmul(pq[:], wq_sb[:], xT[:, sl])
                    nc.vector.tensor_copy(qTc[g][:], pq[:])
                    pv = psA.tile([128, 512], f32, tag="proj", name=f"pv{g}")
                    pvv = pv[:, :4 * H].rearrange("p (n h) -> p n h", n=4)
                    for u in range(4):
                        i = 4 * g + u
                        nc.tensor.matmul(
                            pvv[:, u, :], xT[:, 128 * i:128 * (i + 1)], wv_sb[:]
                        )
                    nc.vector.tensor_copy(vSc[g][:, :, :H], pvv[:])
                    nc.vector.memset(vSc[g][:, :, H].bitcast(f32), 1.0)

                def q_ap(qb):
                    return qTc[qb // 2][:, (qb % 2) * W:(qb % 2) * W + W]

                def emit_st_exp(idx):
                    qb, nj, g0, gn = groups[idx]
                    st = psS.tile([128, GROUP * W], f32, tag="st")
                    for jj in range(gn):
                        j = g0 + jj
                        nc.tensor.matmul(
                            st[:, jj * W:(jj + 1) * W],
                            kTc[j // 4][:, (j % 4) * 128:(j % 4) * 128 + 128],
                            q_ap(qb),
                        )
                    pt = ptp.tile([128, GROUP * W], mm_dt, tag="pt")
                    pt_tiles[idx] = pt
                    nc.scalar.activation(
                        pt[:, :gn * W], st[:, :gn * W], Exp, scale=SCALE
                    )
                    if g0 + gn == nj:  # group holding the 2 diagonal tiles
                        off = (nj - 2 - g0) * W
                        nc.vector.tensor_mul(
                            pt[:, off:off + 2 * W], pt[:, off:off + 2 * W], dm_sb[:]
                        )

                def emit_pv(idx):
                    qb, nj, g0, gn = groups[idx]
                    if g0 == 0:
                        o_tiles[qb] = psO.tile([H + 1, W], f32, tag="o", name=f"o{qb}")
                    pt = pt_tiles.pop(idx)
                    for jj in range(gn):
                        j = g0 + jj
                        nc.tensor.matmul(
                            o_tiles[qb][:],
                            vSc[j // 4][:, j % 4, :],
                            pt[:, jj * W:(jj + 1) * W],
                            start=(j == 0),
                            stop=(j == nj - 1),
                        )
                    if g0 + gn == nj:
                        nc.vector.tensor_copy(oTc[qb][:], o_tiles.pop(qb)[:])
                        nc.sync.dma_start(
                            outT.ap()[:, W * qb:W * (qb + 1)], oTc[qb][:]
                        )

                # emission plan: proj chunk g lands right before the first
                # query block needing it (qb0:proj0, qb2:proj1, qb4:proj2,
                # qb6:proj3); ST/exp runs one group ahead of PV.
                qb_first_idx = {}
                for i2, grp in enumerate(groups):
                    qb_first_idx.setdefault(grp[0], i2)
                proj_at = {qb_first_idx[0]: 0, qb_first_idx[2]: 1,
                           qb_first_idx[4]: 2, qb_first_idx[6]: 3}

                emit_proj(proj_at.pop(qb_first_idx[0]))
                for idx in range(len(groups)):
                    if idx in proj_at:
                        emit_proj(proj_at.pop(idx))
                    emit_st_exp(idx)
                    if idx > 0:
                        emit_pv(idx - 1)
                emit_pv(len(groups) - 1)

    nc.finalize()
    return nc


def _get_nc(prec: str):
    if prec not in _CACHE:
        _CACHE[prec] = _build(prec)
    return _CACHE[prec]


def _host_inputs(Wq, Wk, Wv):
    cst = np.zeros((128, 784), np.float32)
    cst[:, 0:H] = Wq
    cst[:, 128:128 + H] = Wk
    cst[:D, 256:256 + H] = Wv
    r = np.arange(128)[:, None]
    c = np.arange(W)[None, :]
    cst[:, 272:272 + W] = (r <= c).astype(np.float32)
    cst[:, 272 + W:272 + 2 * W] = (128 + r <= c).astype(np.float32)
    return cst


def kernel(inpEmb, Wq, Wk, Wv):
    from concourse.bass_utils import run_bass_kernel_spmd

    prec = os.environ.get("ATT_PREC", "f32r")
    nc = _get_nc(prec)
    cst = _host_inputs(Wq, Wk, Wv)
    x = np.asarray(inpEmb, dtype=np.float32)
    in_maps = [
        {"xT": np.ascontiguousarray(x[b].T), "cst": cst} for b in range(B)
    ]
    br = None
    for attempt in range(3):
        try:
            br = run_bass_kernel_spmd(nc, in_maps, list(range(B)))
            break
        except Exception:
            if attempt == 2:
                raise
    out = np.empty((B, T, H), np.float32)
    for b in range(B):
        oT = br.results[b]["outT"]
        out[b] = (oT[:H] / oT[H:H + 1]).T
    return out


# revision 29
# speedup vs baseline: 10161.8365x; 9024.0935x over previous
"""Causal single-head attention (B=8, T=2048, D=128, H=16) on 8 Trainium2 cores.

Strategy: data-parallel over batch (1 batch element per NeuronCore). Per core:
  - x arrives pre-transposed from the host as xT [D, T] (contiguous DMA).
  - Project qT/kT = (Wq/Wk)^T @ xT with head dim zero-padded 16->128 so every
    matmul contracts over K=128; v tiles [128, 17] carry a ones column so the
    softmax denominator falls out of the PV matmul for free.
  - Scores are computed TRANSPOSED: ST[keys, queries] = kT_j^T @ qT_block, so
    exp(ST) (ACT, scale=1/4 folded in) is directly the PV stationary-side
    operand -- no per-tile transposes of the probability matrix.
  - PV: O'T[17, W] += V'_j^T @ PT_j accumulated in PSUM over key tiles.
  - Causal masking: only key tiles j with 128*j < W*(qb+1) are computed; the
    two diagonal tiles get a multiplicative 0/1 mask after exp.
  - ST-matmul groups are software-pipelined one group ahead of the PV matmuls
    so TensorE streams scores for group g+1 while ScalarE exponentiates group
    g and TensorE then immediately consumes it for PV.
Output per core: outT [17, T] (16 unnormalized head dims + the exp-sum row).
Host divides and transposes during the gather step.
"""

import os

import numpy as np

B, T, D, H = 8, 2048, 128, 16
NT = T // 128        # 16 key tiles of 128
W = 256              # query block width (fp32r needs moving dim >= 256)
NQB = T // W         # 8 query blocks
GROUP = 4            # key tiles per exp call ([128, GROUP*W] = 2 PSUM banks)
SCALE = H ** -0.5

_CACHE = {}


def _build(prec: str):
    import concourse.mybir as mybir
    import concourse.tile as tile
    from concourse import bacc

    f32 = mybir.dt.float32
    mm_dt = {"f32r": mybir.dt.float32r, "f16": mybir.dt.float16, "f32": f32}[prec]
    Exp = mybir.ActivationFunctionType.Exp

    nc = bacc.Bacc()
    xT_d = nc.declare_dram_parameter("xT", [D, T], mm_dt, isOutput=False)
    # packed constants: wq[0:128] | wk[128:256] | wv[256:272]
    cst = nc.declare_dram_parameter("cst", [128, 272], mm_dt, isOutput=False)
    outT = nc.declare_dram_parameter("outT", [H + 1, T], f32, isOutput=True)

    with tile.TileContext(nc) as tc:
        with tc.tile_pool(name="sb", bufs=1) as sb:
            # ---- persistent SBUF buffers ----
            cst_sb = sb.tile([128, 272], mm_dt, tag="cst")
            nc.gpsimd.dma_start(cst_sb[:], cst.ap())  # SWDGE: parallel to x
            wq_sb = cst_sb[:, 0:128]
            wk_sb = cst_sb[:, 128:256]
            wv_sb = cst_sb[:, 256:272]
            # diagonal masks generated on the idle GPSIMD engine:
            # dm[:, c] over the two diagonal key tiles (see _host_inputs docs)
            mdt = mm_dt if prec == "f16" else f32
            dm_full = sb.tile([128, 2 * W], mdt, tag="dm")
            nc.gpsimd.memset(dm_full[:], 1.0)
            nc.gpsimd.affine_select(
                out=dm_full[:, :W], in_=dm_full[:, :W],
                compare_op=mybir.AluOpType.is_ge, fill=0.0,
                base=0, pattern=[[1, W]], channel_multiplier=-1,
            )
            nc.gpsimd.affine_select(
                out=dm_full[:, W:], in_=dm_full[:, W:],
                compare_op=mybir.AluOpType.is_ge, fill=0.0,
                base=-128, pattern=[[1, W]], channel_multiplier=-1,
            )
            dm_sb = dm_full

            CH = [(0, 256), (256, 256), (512, 512), (1024, 512), (1536, 512)]
            xT = sb.tile([128, T], mm_dt, tag="xT")           # [d, t]
            for c0, cw in CH:
                nc.sync.dma_start(
                    xT[:, c0:c0 + cw], xT_d.ap()[:, c0:c0 + cw]
                )

            warm = sb.tile([1, 2], f32, tag="warm")
            nc.vector.memset(warm[:, 0:1], 0.0)
            nc.scalar.activation(warm[:, 1:2], warm[:, 0:1], Exp)

            qTc = [sb.tile([128, cw], mm_dt, tag=f"qT{g}", name=f"qT{g}")
                   for g, (c0, cw) in enumerate(CH)]
            kTc = [sb.tile([128, cw], mm_dt, tag=f"kT{g}", name=f"kT{g}")
                   for g, (c0, cw) in enumerate(CH)]
            vSc = [sb.tile([128, cw // 128, H + 1], mm_dt, tag=f"vS{g}", name=f"vS{g}")
                   for g, (c0, cw) in enumerate(CH)]

            def chunk_of(col):  # chunk index, offset for column `col`
                for g, (c0, cw) in enumerate(CH):
                    if c0 <= col < c0 + cw:
                        return g, col - c0
                raise AssertionError(col)
            oTc = [sb.tile([H + 1, W], f32, tag=f"oT{qb}", name=f"oT{qb}") for qb in range(NQB)]

            groups = []
            for qb in range(NQB):
                nj = (W * (qb + 1)) // 128
                qb_groups = [
                    (qb, nj, g0, min(GROUP, nj - g0))
                    for g0 in range(0, nj, GROUP)
                ]
                # descending start: the diagonal (masked) group is consumed
                # first, keeping the exp->mask->PV chain off the qb tail
                groups.extend(reversed(qb_groups))

            with (
                tc.tile_pool(name="psS", bufs=3, space="PSUM") as psS,
                tc.tile_pool(name="psO", bufs=2, space="PSUM") as psO,
                tc.tile_pool(name="pt", bufs=8) as ptp,
            ):
                o_tiles = {}
                pt_tiles = {}

                def emit_proj(g):
                    c0, cw = CH[g]
                    nt = cw // 128
                    sl = slice(c0, c0 + cw)
                    if 2 * cw + nt * H <= GROUP * W:
                        pp = psS.tile([128, GROUP * W], f32, tag="st", name=f"pp{g}")
                        pk, pq, pv = pp[:, :cw], pp[:, cw:2 * cw], pp[:, 2 * cw:2 * cw + nt * H]
                    else:
                        pp = psS.tile([128, GROUP * W], f32, tag="st", name=f"ppa{g}")
                        pp2 = psS.tile([128, GROUP * W], f32, tag="st", name=f"ppb{g}")
                        pk, pv = pp[:, :cw], pp[:, cw:cw + nt * H]
                        pq = pp2[:, :cw]
                    nc.tensor.matmul(pk, wk_sb[:], xT[:, sl])
                    nc.vector.tensor_copy(kTc[g][:], pk)
                    nc.tensor.matmul(pq, wq_sb[:], xT[:, sl])
                    nc.vector.tensor_copy(qTc[g][:], pq)
                    pvv = pv.rearrange("p (n h) -> p n h", n=nt)
                    for u in range(nt):
                        i = (c0 // 128) + u
                        nc.tensor.matmul(
                            pvv[:, u, :], xT[:, 128 * i:128 * (i + 1)], wv_sb[:]
                        )
                    nc.vector.tensor_copy(vSc[g][:, :, :H], pvv[:])
                    if prec == "f32r":
                        nc.vector.memset(vSc[g][:, :, H].bitcast(f32), 1.0)
                    else:
                        nc.vector.memset(vSc[g][:, :, H], 1.0)

                def q_ap(qb):
                    g, off = chunk_of(W * qb)
                    return qTc[g][:, off:off + W]

                def emit_st_exp(idx):
                    qb, nj, g0, gn = groups[idx]
                    st = psS.tile([128, GROUP * W], f32, tag="st")
                    for jj in range(gn):
                        j = g0 + jj
                        kg, koff = chunk_of(128 * j)
                        nc.tensor.matmul(
                            st[:, jj * W:(jj + 1) * W],
                            kTc[kg][:, koff:koff + 128],
                            q_ap(qb),
                        )
                    pt = ptp.tile([128, GROUP * W], mm_dt, tag="pt")
                    pt_tiles[idx] = pt
                    nc.scalar.activation(
                        pt[:, :gn * W], st[:, :gn * W], Exp, scale=SCALE
                    )
                    if g0 + gn == nj:  # group holding the 2 diagonal tiles
                        off = (nj - 2 - g0) * W
                        nc.vector.tensor_mul(
                            pt[:, off:off + 2 * W], pt[:, off:off + 2 * W], dm_sb[:]
                        )

                def emit_pv(idx):
                    qb, nj, g0, gn = groups[idx]
                    first_emitted = g0 + GROUP >= nj   # diag group comes first
                    last_emitted = g0 == 0
                    if first_emitted:
                        o_tiles[qb] = psO.tile([H + 1, W], f32, tag="o", name=f"o{qb}")
                    pt = pt_tiles.pop(idx)
                    for jj in range(gn):
                        j = g0 + jj
                        vg, voff = chunk_of(128 * j)
                        nc.tensor.matmul(
                            o_tiles[qb][:],
                            vSc[vg][:, voff // 128, :],
                            pt[:, jj * W:(jj + 1) * W],
                            start=(first_emitted and jj == 0),
                            stop=(last_emitted and jj == gn - 1),
                        )
                    if last_emitted:
                        nc.vector.tensor_copy(oTc[qb][:], o_tiles.pop(qb)[:])
                        nc.sync.dma_start(
                            outT.ap()[:, W * qb:W * (qb + 1)], oTc[qb][:]
                        )

                # emission plan: proj chunk g lands right before the first
                # query block needing it (qb0:proj0, qb2:proj1, qb4:proj2,
                # qb6:proj3); ST/exp runs one group ahead of PV.
                qb_first_idx = {}
                for i2, grp in enumerate(groups):
                    qb_first_idx.setdefault(grp[0], i2)
                proj_at = {qb_first_idx[0]: [0, 1], qb_first_idx[1]: [2],
                           qb_first_idx[2]: [3], qb_first_idx[4]: [4]}

                n = len(groups)
                for g in proj_at.pop(qb_first_idx[0]):
                    emit_proj(g)
                pend = []
                for idx in range(n):
                    for g in proj_at.pop(idx, []):
                        emit_proj(g)
                    emit_st_exp(idx)
                    pend.append(idx)
                    depth = 4 if idx < n - 4 else max(1, n - 1 - idx)
                    while len(pend) > depth:
                        emit_pv(pend.pop(0))
                while pend:
                    emit_pv(pend.pop(0))

    nc.finalize()
    return nc


def _get_nc(prec: str):
    if prec not in _CACHE:
        _CACHE[prec] = _build(prec)
    return _CACHE[prec]


def _host_inputs(Wq, Wk, Wv):
    Wq, Wk, Wv = (np.asarray(w, dtype=np.float32) for w in (Wq, Wk, Wv))
    cst = np.zeros((128, 272), np.float32)
    cst[:, 0:H] = Wq
    cst[:, 128:128 + H] = Wk
    cst[:D, 256:256 + H] = Wv
    return cst


def kernel(inpEmb, Wq, Wk, Wv):
    from concourse.bass_utils import run_bass_kernel_spmd

    prec = os.environ.get("ATT_PREC", "f32r")
    nc = _get_nc(prec)
    np_dt = np.float16 if prec == "f16" else np.float32
    cst = _host_inputs(Wq, Wk, Wv).astype(np_dt)
    x = np.asarray(inpEmb, dtype=np.float32)
    in_maps = [
        {"xT": np.ascontiguousarray(x[b].T.astype(np_dt)), "cst": cst}
        for b in range(B)
    ]
    br = None
    for attempt in range(3):
        try:
            br = run_bass_kernel_spmd(nc, in_maps, list(range(B)))
            break
        except Exception:
            if attempt == 2:
                raise
    out = np.empty((B, T, H), np.float32)
    for b in range(B):
        oT = br.results[b]["outT"]
        out[b] = (oT[:H] / oT[H:H + 1]).T
    return out


# revision 51
# speedup vs baseline: 10411.1775x; 1.0245x over previous
"""Causal single-head attention (B=8, T=2048, D=128, H=16) on 8 Trainium2 cores.

Strategy: data-parallel over batch (1 batch element per NeuronCore). Per core:
  - x arrives pre-transposed from the host as xT [D, T] (contiguous DMA).
  - Project qT/kT = (Wq/Wk)^T @ xT with head dim zero-padded 16->128 so every
    matmul contracts over K=128; v tiles [128, 17] carry a ones column so the
    softmax denominator falls out of the PV matmul for free.
  - Scores are computed TRANSPOSED: ST[keys, queries] = kT_j^T @ qT_block, so
    exp(ST) (ACT, scale=1/4 folded in) is directly the PV stationary-side
    operand -- no per-tile transposes of the probability matrix.
  - PV: O'T[17, W] += V'_j^T @ PT_j accumulated in PSUM over key tiles.
  - Causal masking: only key tiles j with 128*j < W*(qb+1) are computed; the
    two diagonal tiles get a multiplicative 0/1 mask after exp.
  - ST-matmul groups are software-pipelined up to 4 groups ahead of the PV
    matmuls (tapered at the end) so TensorE streams future scores while
    ScalarE exponentiates and TensorE consumes finished groups for PV;
    ScalarE (the exp bottleneck, ~1 elem/lane/cycle) stays saturated.
Output per core: outT [17, T] (16 unnormalized head dims + the exp-sum row).
Host divides and transposes during the gather step.
"""

import os

import numpy as np

B, T, D, H = 8, 2048, 128, 16
NT = T // 128        # 16 key tiles of 128
W = 256              # query block width (fp32r needs moving dim >= 256)
NQB = T // W         # 8 query blocks
GROUP = 4            # key tiles per exp call ([128, GROUP*W] = 2 PSUM banks)
SCALE = H ** -0.5

_CACHE = {}


def _build(prec: str):
    import concourse.mybir as mybir
    import concourse.tile as tile
    from concourse import bacc

    f32 = mybir.dt.float32
    mm_dt = {"f32r": mybir.dt.float32r, "f16": mybir.dt.float16, "f32": f32}[prec]
    Exp = mybir.ActivationFunctionType.Exp

    nc = bacc.Bacc()
    xT_d = nc.declare_dram_parameter("xT", [D, T], mm_dt, isOutput=False)
    # packed constants: wq[0:128] | wk[128:256] | wv[256:272]
    cst = nc.declare_dram_parameter("cst", [128, 272], mm_dt, isOutput=False)
    outT = nc.declare_dram_parameter("outT", [H + 1, T], f32, isOutput=True)

    with tile.TileContext(nc) as tc:
        with tc.tile_pool(name="sb", bufs=1) as sb:
            # ---- persistent SBUF buffers ----
            cst_sb = sb.tile([128, 272], mm_dt, tag="cst")
            nc.gpsimd.dma_start(cst_sb[:], cst.ap())  # SWDGE: parallel to x
            wq_sb = cst_sb[:, 0:128]
            wk_sb = cst_sb[:, 128:256]
            wv_sb = cst_sb[:, 256:272]
            # diagonal masks generated on the idle GPSIMD engine:
            # dm[:, c] over the two diagonal key tiles (see _host_inputs docs)
            mdt = mm_dt if prec == "f16" else f32
            dm_full = sb.tile([128, 2 * W], mdt, tag="dm")
            nc.gpsimd.memset(dm_full[:], 1.0)
            nc.gpsimd.affine_select(
                out=dm_full[:, :W], in_=dm_full[:, :W],
                compare_op=mybir.AluOpType.is_ge, fill=0.0,
                base=0, pattern=[[1, W]], channel_multiplier=-1,
            )
            nc.gpsimd.affine_select(
                out=dm_full[:, W:], in_=dm_full[:, W:],
                compare_op=mybir.AluOpType.is_ge, fill=0.0,
                base=-128, pattern=[[1, W]], channel_multiplier=-1,
            )
            dm_sb = dm_full

            CH = [(0, 256), (256, 256), (512, 512), (1024, 512), (1536, 512)]
            xT = sb.tile([128, T], mm_dt, tag="xT")           # [d, t]
            for c0, cw in CH:
                nc.sync.dma_start(
                    xT[:, c0:c0 + cw], xT_d.ap()[:, c0:c0 + cw]
                )

            warm = sb.tile([1, 2], f32, tag="warm")
            nc.vector.memset(warm[:, 0:1], 0.0)
            nc.scalar.activation(warm[:, 1:2], warm[:, 0:1], Exp)
            # PE warm-up: dummy matmuls during the input DMA keep the HAM
            # activity monitor busy so real matmuls start at full clock.
            wdum = sb.tile([128, 512], f32, tag="wdum")
            nc.vector.memset(wdum[:], 0.0)

            qTc = [sb.tile([128, cw], mm_dt, tag=f"qT{g}", name=f"qT{g}")
                   for g, (c0, cw) in enumerate(CH)]
            kTc = [sb.tile([128, cw], mm_dt, tag=f"kT{g}", name=f"kT{g}")
                   for g, (c0, cw) in enumerate(CH)]
            vSc = [sb.tile([128, cw // 128, H + 1], mm_dt, tag=f"vS{g}", name=f"vS{g}")
                   for g, (c0, cw) in enumerate(CH)]

            def chunk_of(col):  # chunk index, offset for column `col`
                for g, (c0, cw) in enumerate(CH):
                    if c0 <= col < c0 + cw:
                        return g, col - c0
                raise AssertionError(col)
            oTc = [sb.tile([H + 1, W], f32, tag=f"oT{qb}", name=f"oT{qb}") for qb in range(NQB)]

            groups = []
            for qb in range(NQB):
                nj = (W * (qb + 1)) // 128
                qb_groups = [
                    (qb, nj, g0, min(GROUP, nj - g0))
                    for g0 in range(0, nj, GROUP)
                ]
                # descending start: the diagonal (masked) group is consumed
                # first, keeping the exp->mask->PV chain off the qb tail
                groups.extend(reversed(qb_groups))

            with (
                tc.tile_pool(name="psS", bufs=3, space="PSUM") as psS,
                tc.tile_pool(name="psO", bufs=2, space="PSUM") as psO,
                tc.tile_pool(name="pt", bufs=8) as ptp,
            ):
                o_tiles = {}
                pt_tiles = {}

                def emit_proj(g):
                    c0, cw = CH[g]
                    nt = cw // 128
                    sl = slice(c0, c0 + cw)
                    if 2 * cw + nt * H <= GROUP * W:
                        pp = psS.tile([128, GROUP * W], f32, tag="st", name=f"pp{g}")
                        pk, pq, pv = pp[:, :cw], pp[:, cw:2 * cw], pp[:, 2 * cw:2 * cw + nt * H]
                    else:
                        pp = psS.tile([128, GROUP * W], f32, tag="st", name=f"ppa{g}")
                        pp2 = psS.tile([128, GROUP * W], f32, tag="st", name=f"ppb{g}")
                        pk, pv = pp[:, :cw], pp[:, cw:cw + nt * H]
                        pq = pp2[:, :cw]
                    nc.tensor.matmul(pk, wk_sb[:], xT[:, sl])
                    if g <= 2:  # ramp phase: ACT has idle capacity
                        nc.scalar.copy(kTc[g][:], pk)
                    else:
                        nc.vector.tensor_copy(kTc[g][:], pk)
                    nc.tensor.matmul(pq, wq_sb[:], xT[:, sl])
                    nc.vector.tensor_copy(qTc[g][:], pq)
                    pvv = pv.rearrange("p (n h) -> p n h", n=nt)
                    for u in range(nt):
                        i = (c0 // 128) + u
                        nc.tensor.matmul(
                            pvv[:, u, :], xT[:, 128 * i:128 * (i + 1)], wv_sb[:]
                        )
                    nc.vector.tensor_copy(vSc[g][:, :, :H], pvv[:])
                    if prec == "f32r":
                        nc.vector.memset(vSc[g][:, :, H].bitcast(f32), 1.0)
                    else:
                        nc.vector.memset(vSc[g][:, :, H], 1.0)

                def q_ap(qb):
                    g, off = chunk_of(W * qb)
                    return qTc[g][:, off:off + W]

                def emit_st_exp(idx):
                    qb, nj, g0, gn = groups[idx]
                    st = psS.tile([128, GROUP * W], f32, tag="st")
                    for jj in range(gn):
                        j = g0 + jj
                        kg, koff = chunk_of(128 * j)
                        nc.tensor.matmul(
                            st[:, jj * W:(jj + 1) * W],
                            kTc[kg][:, koff:koff + 128],
                            q_ap(qb),
                        )
                    pt = ptp.tile([128, GROUP * W], mm_dt, tag="pt")
                    pt_tiles[idx] = pt
                    nc.scalar.activation(
                        pt[:, :gn * W], st[:, :gn * W], Exp, scale=SCALE
                    )
                    if g0 + gn == nj:  # group holding the 2 diagonal tiles
                        off = (nj - 2 - g0) * W
                        nc.vector.tensor_mul(
                            pt[:, off:off + 2 * W], pt[:, off:off + 2 * W], dm_sb[:]
                        )

                def emit_pv(idx):
                    qb, nj, g0, gn = groups[idx]
                    first_emitted = g0 + GROUP >= nj   # diag group comes first
                    last_emitted = g0 == 0
                    if first_emitted:
                        o_tiles[qb] = psO.tile([H + 1, W], f32, tag="o", name=f"o{qb}")
                    pt = pt_tiles.pop(idx)
                    for jj in range(gn):
                        j = g0 + jj
                        vg, voff = chunk_of(128 * j)
                        nc.tensor.matmul(
                            o_tiles[qb][:],
                            vSc[vg][:, voff // 128, :],
                            pt[:, jj * W:(jj + 1) * W],
                            start=(first_emitted and jj == 0),
                            stop=(last_emitted and jj == gn - 1),
                        )
                    if last_emitted:
                        nc.vector.tensor_copy(oTc[qb][:], o_tiles.pop(qb)[:])
                        nc.sync.dma_start(
                            outT.ap()[:, W * qb:W * (qb + 1)], oTc[qb][:]
                        )

                # emission plan: projection chunks land just before the
                # first query block that needs them; ST/exp runs up to 4
                # groups ahead of PV (tapered near the end).
                # assign each proj chunk to the first group that needs it
                # projection-chunk emission slots (group indices). Chunks
                # MUST be emitted at or before the first group that reads
                # them: Tile tracks dependencies by trace order, so a
                # consumer emitted before its producer silently reads stale
                # SBUF (verified: gives nondeterministic garbage).
                proj_at = {0: [0, 1], 1: [2], 2: [3], 6: [4]}
                first_need = {}
                for i2, (qb, nj, g0, gn) in enumerate(groups):
                    need = {chunk_of(W * qb)[0]}
                    need.update(chunk_of(128 * j)[0] for j in range(g0, g0 + gn))
                    for g in need:
                        first_need.setdefault(g, i2)
                for slot, gs in proj_at.items():
                    for g in gs:
                        assert slot <= first_need[g], (slot, g, first_need[g])

                n = len(groups)
                pdum = psS.tile([128, GROUP * W], f32, tag="st", name="pdum")
                for r in range(4):
                    nc.tensor.matmul(
                        pdum[:, :512], wdum[:, :128].bitcast(mm_dt),
                        wdum[:].bitcast(mm_dt),
                    )
                for g in proj_at.pop(0, []):
                    emit_proj(g)
                pend = []
                for idx in range(n):
                    for g in proj_at.pop(idx, []):
                        emit_proj(g)
                    emit_st_exp(idx)
                    pend.append(idx)
                    depth = 4 if idx < n - 4 else max(1, n - 1 - idx)
                    while len(pend) > depth:
                        emit_pv(pend.pop(0))
                while pend:
                    emit_pv(pend.pop(0))

    nc.finalize()
    return nc


def _get_nc(prec: str):
    if prec not in _CACHE:
        _CACHE[prec] = _build(prec)
    return _CACHE[prec]


def _host_inputs(Wq, Wk, Wv):
    Wq, Wk, Wv = (np.asarray(w, dtype=np.float32) for w in (Wq, Wk, Wv))
    cst = np.zeros((128, 272), np.float32)
    cst[:, 0:H] = Wq
    cst[:, 128:128 + H] = Wk
    cst[:D, 256:256 + H] = Wv
    return cst


def kernel(inpEmb, Wq, Wk, Wv):
    from concourse.bass_utils import run_bass_kernel_spmd

    prec = os.environ.get("ATT_PREC", "f32r")
    nc = _get_nc(prec)
    np_dt = np.float16 if prec == "f16" else np.float32
    cst = _host_inputs(Wq, Wk, Wv).astype(np_dt)
    x = np.asarray(inpEmb, dtype=np.float32)
    in_maps = [
        {"xT": np.ascontiguousarray(x[b].T.astype(np_dt)), "cst": cst}
        for b in range(B)
    ]
    br = None
    for attempt in range(3):
        try:
            br = run_bass_kernel_spmd(nc, in_maps, list(range(B)))
            break
        except Exception:
            if attempt == 2:
                raise
    out = np.empty((B, T, H), np.float32)
    for b in range(B):
        oT = br.results[b]["outT"]
        out[b] = (oT[:H] / oT[H:H + 1]).T
    return out
